# revision 17
# baseline (speedup 1.0000x reference)
"""Cross-attention layer (B=2, LQ=1024, LKV=4096, H=1024, NH=16) on 8 trn2 cores.

Sharding: Megatron-style over heads — each core owns 2 heads (128 channels of
q/k/v and 128 input channels of o_w). Scores output is head-sharded (disjoint
per core). Output projection produces partial sums that are ReduceScattered
over flat tokens; residual + LayerNorm run on the local token slice.

Device-side layouts (per core):
  QTA_h [65, B*LQ]   rows 0-63: q_h^T (pre-scaled by 1/8), row 64: -rowmax(scores)
  KTA_h [65, LKV]    rows 0-63: k_h^T, row 64: ones      (per-batch, reused)
  VO    [128, 2*32*65]  per (h, kv-tile) [128, 65] v-natural + ones column
                        (ones col at 64 for h0, at 0 for h1)
  scores^A [q,kv] = QTA[0:64].T @ KTA[0:64]           -> raw scores output
  scores^B [kv,q] = KTA[0:65].T @ QTA[0:65] = s - m   -> exp -> ctx^T accum
  ctx^T via VO.T @ expB with built-in ones row giving softmax denominators.

Mask handling: softmax masking via ACT exp bias (per-kv additive -1e30);
raw-scores masking applied on host (exact -inf like the reference).
"""
import sys
import types

import numpy as np

import concourse.bacc as bacc
import concourse.bass as bass
import concourse.tile as tile
from concourse import mybir
import concourse.bass_utils as bass_utils
from concourse.bass_utils import run_bass_kernel_spmd
from concourse.masks import make_identity

dt = mybir.dt
AF = mybir.ActivationFunctionType
AX = mybir.AxisListType
OP = mybir.AluOpType

B, LQ, LKV, H, NH, HD = 2, 1024, 4096, 1024, 16, 64
NCORES = 8
CL = H // NCORES          # 128 local channels
NHL = NH // NCORES        # 2 local heads
TQ = B * LQ               # 2048 flat q tokens
TLOC = TQ // NCORES       # 256 tokens per core (128 per batch)
NKV = LKV // 128          # 32 kv-tiles per batch
EPS = 1e-12
NEG = -1.0e30


def _install_ntff_hook():
    """Agent image's antenv lacks axon_hooks; provide it so trace=True works."""
    try:
        import antenv.axon_hooks  # noqa: F401
        return
    except ImportError:
        pass
    try:
        import trn_agent_boot.trn_boot as tb
        hook = tb._ntff_profile_via_ctypes("/opt/axon/libaxon_pjrt.so")
    except Exception:
        hook = None
    mod = types.ModuleType("antenv.axon_hooks")
    mod.get_axon_ntff_profile_hook = lambda: hook
    mod.set_axon_ntff_profile_hook = lambda h: None
    sys.modules["antenv.axon_hooks"] = mod
    bass_utils.upload_artifacts = lambda tmpdir: tmpdir


def build_nc(f32r=True):
    rdt = dt.float32r if f32r else dt.float32
    f32 = dt.float32
    nc = bacc.Bacc("TRN2", target_bir_lowering=False, debug=False,
                   num_devices=NCORES)

    # ---- dram parameters (per-core inputs) ----
    hT_d = nc.declare_dram_parameter("hT", [B, H, LQ], rdt, isOutput=False)
    encT_d = nc.declare_dram_parameter("encT", [B, H, LKV], rdt, isOutput=False)
    qwT_d = nc.declare_dram_parameter("qwT", [H, CL], rdt, isOutput=False)
    kwT_d = nc.declare_dram_parameter("kwT", [H, CL], rdt, isOutput=False)
    vwT_d = nc.declare_dram_parameter("vwT", [H, CL], rdt, isOutput=False)
    owT_d = nc.declare_dram_parameter("owT", [CL, H], rdt, isOutput=False)
    onesr_d = nc.declare_dram_parameter("onesr", [1, LKV], rdt, isOutput=False)
    qb8_d = nc.declare_dram_parameter("qb8", [1, CL], f32, isOutput=False)
    kb_d = nc.declare_dram_parameter("kb", [1, CL], f32, isOutput=False)
    vb_d = nc.declare_dram_parameter("vb", [1, CL], f32, isOutput=False)
    ob8_d = nc.declare_dram_parameter("ob8", [1, H], f32, isOutput=False)
    mbT_d = nc.declare_dram_parameter("mbT", [128, B * NKV], f32, isOutput=False)
    hres_d = nc.declare_dram_parameter("hres", [B, 128, H], f32, isOutput=False)
    lnw_d = nc.declare_dram_parameter("lnw", [1, H], f32, isOutput=False)
    lnb_d = nc.declare_dram_parameter("lnb", [1, H], f32, isOutput=False)

    # ---- outputs ----
    scores_d = nc.declare_dram_parameter(
        "scores_loc", [B, NHL, LQ, LKV], f32, isOutput=True)
    out_d = nc.declare_dram_parameter("out_loc", [B, 128, H], f32, isOutput=True)
    dbg_negm_d = nc.declare_dram_parameter(
        "dbg_negm", [B, NHL, 8, 128], rdt, isOutput=True)
    dbg_den_d = nc.declare_dram_parameter(
        "dbg_den", [B, NHL, 2, 512], f32, isOutput=True)
    dbg_part_d = nc.declare_dram_parameter(
        "dbg_part", [B, 128, H], f32, isOutput=True)

    # ---- internal dram (collective in/out, per batch) ----
    part_d = [nc.dram_tensor(f"part{b}", [LQ, H], f32) for b in range(B)]
    rsout_d = [nc.dram_tensor(f"rsout{b}", [128, H], f32) for b in range(B)]

    with tile.TileContext(nc) as tc:
        with (
            tc.tile_pool(name="per", bufs=1) as per,
            tc.tile_pool(name="kvper", bufs=1) as kvper,
            tc.tile_pool(name="acts", bufs=8) as acts,
            tc.tile_pool(name="stg", bufs=3) as stg,
            tc.tile_pool(name="expp", bufs=3) as expp,
            tc.tile_pool(name="sm", bufs=1) as sm,
            tc.tile_pool(name="lnp", bufs=1) as lnp,
            tc.tile_pool(name="psM", bufs=3, space="PSUM") as psM,
            tc.tile_pool(name="psA", bufs=2, space="PSUM") as psA,
            tc.tile_pool(name="psB", bufs=2, space="PSUM") as psB,
            tc.tile_pool(name="psC", bufs=1, space="PSUM") as psC,
        ):
            # ================= constants =================
            qwT_sb = per.tile([128, H], rdt)     # [i-in-tile, k-major: 8 x 128c]
            kwT_sb = per.tile([128, H], rdt)
            vwT_sb = per.tile([128, H], rdt)
            for w_sb, w_d in ((qwT_sb, qwT_d), (kwT_sb, kwT_d), (vwT_sb, vwT_d)):
                nc.sync.dma_start(
                    out=w_sb[:].rearrange("p (k c) -> p k c", c=CL),
                    in_=w_d[:, :].rearrange("(k p) c -> p k c", p=128),
                )

            qb8_sb = per.tile([1, CL], f32)
            kb_sb = per.tile([1, CL], f32)
            vb_sb = per.tile([1, CL], f32)
            ob8_sb = per.tile([1, H], f32)
            mbT_sb = per.tile([128, B * NKV], f32)
            lnw_sb = per.tile([1, H], f32)
            lnb_sb = per.tile([1, H], f32)
            for t, d in ((qb8_sb, qb8_d), (kb_sb, kb_d), (vb_sb, vb_d),
                         (ob8_sb, ob8_d), (mbT_sb, mbT_d), (lnw_sb, lnw_d),
                         (lnb_sb, lnb_d)):
                nc.sync.dma_start(out=t[:], in_=d[:, :])

            ones1 = per.tile([1, 128], f32)
            nc.vector.memset(ones1[:], 1.0)
            ones_row = per.tile([1, 512], f32)
            nc.vector.memset(ones_row[:], 1.0)
            ident = per.tile([128, 128], f32)
            make_identity(nc, ident[:])
            epsb = per.tile([128, 1], f32)
            nc.vector.memset(epsb[:], EPS)

            # ln scale/bias broadcast to all partitions via rank-1 matmuls
            lnwB = per.tile([128, H], f32)
            lnbB = per.tile([128, H], f32)
            for src, dst in ((lnw_sb, lnwB), (lnb_sb, lnbB)):
                for half in range(2):
                    ps = psM.tile([128, 512], f32, tag="m")
                    nc.tensor.matmul(ps[:], ones1[:], src[0:1, bass.ts(half, 512)],
                                     start=True, stop=True)
                    nc.scalar.copy(dst[:, bass.ts(half, 512)], ps[:])

            # persistent activations
            QTA = [per.tile([65, TQ], rdt, name=f"qta{h}") for h in range(NHL)]
            ctxT = [per.tile([64, TQ], rdt, name=f"ctxt{h}") for h in range(NHL)]
            owT_h = [per.tile([64, H], rdt, name=f"owt{h}") for h in range(NHL)]
            for h in range(NHL):
                nc.sync.dma_start(out=owT_h[h][:],
                                  in_=owT_d[64 * h:64 * (h + 1), :])

            # ================= Q projection =================
            for b in range(B):
                ht = []
                for k in range(8):
                    t = acts.tile([128, 1024], rdt, tag="act")
                    nc.sync.dma_start(out=t[:], in_=hT_d[b, bass.ts(k, 128), :])
                    ht.append(t)
                for tt in range(2):  # 512-token chunks
                    for h in range(NHL):
                        ps = psM.tile([64, 512], f32, tag="m")
                        for k in range(8):
                            nc.tensor.matmul(
                                ps[:],
                                qwT_sb[:, 128 * k + 64 * h:128 * k + 64 * h + 64],
                                ht[k][:, bass.ts(tt, 512)],
                                start=(k == 0), stop=False)
                        nc.tensor.matmul(ps[:], qb8_sb[0:1, bass.ts(h, 64)],
                                         ones_row[:], start=False, stop=True)
                        nc.scalar.copy(
                            QTA[h][0:64, b * LQ + 512 * tt: b * LQ + 512 * (tt + 1)],
                            ps[:])

            # ============ per-batch: K/V proj, attention, o-proj, RS, LN ========
            for b in range(B):
                KTA = [kvper.tile([65, LKV], rdt, name=f"kta{h}") for h in range(NHL)]
                VO = kvper.tile([128, NHL * NKV * 65], rdt, name="vo")
                # ones row for the augmented K (row 64)
                for h in range(NHL):
                    nc.sync.dma_start(out=KTA[h][64:65, :], in_=onesr_d[0:1, :])

                for j4 in range(4):  # 1024-wide kv chunks
                    et = []
                    for k in range(8):
                        t = acts.tile([128, 1024], rdt, tag="act")
                        nc.sync.dma_start(
                            out=t[:],
                            in_=encT_d[b, bass.ts(k, 128),
                                       1024 * j4:1024 * (j4 + 1)])
                        et.append(t)
                    # K^T
                    for tt in range(2):
                        kv0 = 1024 * j4 + 512 * tt
                        for h in range(NHL):
                            ps = psM.tile([64, 512], f32, tag="m")
                            for k in range(8):
                                nc.tensor.matmul(
                                    ps[:],
                                    kwT_sb[:, 128 * k + 64 * h:128 * k + 64 * h + 64],
                                    et[k][:, bass.ts(tt, 512)],
                                    start=(k == 0), stop=False)
                            nc.tensor.matmul(ps[:], kb_sb[0:1, bass.ts(h, 64)],
                                             ones_row[:], start=False, stop=True)
                            nc.scalar.copy(KTA[h][0:64, kv0:kv0 + 512], ps[:])
                    # V natural [kv, c] + ones columns
                    for v8 in range(8):
                        j = 8 * j4 + v8  # kv-tile index within batch
                        # fp32r matmul dst needs 8B-aligned free offset, so the
                        # h1 v-block starts at column 66 (not 65).
                        ps = psM.tile([128, 131], f32, tag="m")
                        for k in range(8):
                            lhsT = et[k][:, bass.ts(v8, 128)]
                            nc.tensor.matmul(ps[:, 0:64], lhsT,
                                             vwT_sb[:, 128 * k:128 * k + 64],
                                             start=(k == 0), stop=False)
                            nc.tensor.matmul(ps[:, 66:130], lhsT,
                                             vwT_sb[:, 128 * k + 64:128 * (k + 1)],
                                             start=(k == 0), stop=False)
                        nc.tensor.matmul(ps[:, 0:64], ones1[:],
                                         vb_sb[0:1, 0:64], start=False, stop=False)
                        nc.tensor.matmul(ps[:, 66:130], ones1[:],
                                         vb_sb[0:1, 64:128], start=False, stop=False)
                        nc.tensor.matmul(ps[:, 64:65], ones1[:],
                                         ones_row[0:1, 0:1], start=False, stop=False)
                        nc.tensor.matmul(ps[:, 130:131], ones1[:],
                                         ones_row[0:1, 0:1], start=False, stop=True)
                        # per-head [v(64) | 1] tiles
                        nc.scalar.copy(VO[:, 65 * j:65 * (j + 1)], ps[:, 0:65])
                        nc.scalar.copy(VO[:, 65 * (NKV + j):65 * (NKV + j + 1)],
                                       ps[:, 66:131])

                for h in range(NHL):
                    # ---------------- phase A: raw scores + rowmax ----------------
                    negm = sm.tile([128, 8], f32, tag="negm")
                    for qt in range(8):
                        pmax = sm.tile([128, 4], f32, tag="pmax")
                        for kk in range(4):
                            st = stg.tile([128, 1024], f32, tag="stg")
                            for jj in range(2):
                                psa = psA.tile([128, 512], f32, tag="a")
                                nc.tensor.matmul(
                                    psa[:],
                                    QTA[h][0:64,
                                           b * LQ + 128 * qt:b * LQ + 128 * (qt + 1)],
                                    KTA[h][0:64,
                                           1024 * kk + 512 * jj:1024 * kk + 512 * (jj + 1)],
                                    start=True, stop=True)
                                eng = nc.scalar.copy if jj == 0 else nc.vector.tensor_copy
                                eng(st[:, bass.ts(jj, 512)], psa[:])
                            nc.sync.dma_start(
                                out=scores_d[b, h, 128 * qt:128 * (qt + 1),
                                             1024 * kk:1024 * (kk + 1)],
                                in_=st[:])
                            nc.vector.reduce_max(pmax[:, kk:kk + 1], st[:], axis=AX.X)
                        nc.vector.reduce_max(negm[:, qt:qt + 1], pmax[:],
                                             axis=AX.X, negate=True)
                    # transpose -max into QTA row 64 (via PE transpose + sbuf DMA)
                    pst = psM.tile([8, 128], f32, tag="m")
                    nc.tensor.transpose(pst[:], negm[:], ident[:])
                    negmT = sm.tile([8, 128], rdt, tag="negmT")
                    nc.scalar.copy(negmT[:], pst[:])
                    for qt in range(8):
                        nc.sync.dma_start(
                            out=QTA[h][64:65,
                                       b * LQ + 128 * qt:b * LQ + 128 * (qt + 1)],
                            in_=negmT[qt:qt + 1, :])
                    nc.sync.dma_start(out=dbg_negm_d[b, h, :, :], in_=negmT[:])

                    # ---------------- phase B: softmax + ctx^T ----------------
                    for qq in range(2):
                        q0 = b * LQ + 512 * qq
                        ctxps = psC.tile([65, 512], f32, tag="c")
                        for j in range(NKV):
                            psb = psB.tile([128, 512], f32, tag="b")
                            nc.tensor.matmul(
                                psb[:],
                                KTA[h][0:65, bass.ts(j, 128)],
                                QTA[h][0:65, q0:q0 + 512],
                                start=True, stop=True)
                            ex = expp.tile([128, 512], rdt, tag="exp")
                            nc.scalar.activation(
                                ex[:], psb[:], AF.Exp,
                                bias=mbT_sb[:, b * NKV + j:b * NKV + j + 1])
                            nc.tensor.matmul(
                                ctxps[:],
                                VO[:, 65 * (h * NKV + j):65 * (h * NKV + j + 1)],
                                ex[:],
                                start=(j == 0), stop=(j == NKV - 1))
                        # normalize: ctxT rows = ctx / denom.  The denom row sits
                        # at partition 64; bounce it to partition 0 so the rank-1
                        # broadcast matmul has aligned operands.
                        den = sm.tile([128, 512], f32, tag="den")
                        nc.scalar.copy(den[64:65, :], ctxps[64:65, :])
                        den0 = sm.tile([1, 512], f32, tag="den0")
                        nc.sync.dma_start(out=den0[:], in_=den[64:65, :])
                        nc.sync.dma_start(out=dbg_den_d[b, h, qq, :][None, :],
                                          in_=den0[:])
                        recip0 = sm.tile([1, 512], f32, tag="recip0")
                        nc.vector.reciprocal(recip0[:], den0[:])
                        bcp = psM.tile([128, 512], f32, tag="m")
                        nc.tensor.matmul(bcp[0:64, :], ones1[0:1, 0:64],
                                         recip0[0:1, :],
                                         start=True, stop=True)
                        bcs = sm.tile([64, 512], f32, tag="bcs")
                        nc.scalar.copy(bcs[:], bcp[0:64, :])
                        nc.vector.tensor_tensor(
                            out=ctxT[h][0:64, q0:q0 + 512],
                            in0=ctxps[0:64, :],
                            in1=bcs[:],
                            op=OP.mult)

                # ---------------- o-proj partials for this batch ----------------
                for tt in range(8):
                    att = lnp.tile([128, H], f32, tag="att")
                    for half in range(2):
                        pso = psA.tile([128, 512], f32, tag="a")
                        for h in range(NHL):
                            nc.tensor.matmul(
                                pso[:],
                                ctxT[h][0:64,
                                        b * LQ + 128 * tt:b * LQ + 128 * (tt + 1)],
                                owT_h[h][0:64, bass.ts(half, 512)],
                                start=(h == 0), stop=False)
                        nc.tensor.matmul(pso[:], ones1[:],
                                         ob8_sb[0:1, bass.ts(half, 512)],
                                         start=False, stop=True)
                        eng = nc.scalar.copy if half == 0 else nc.vector.tensor_copy
                        eng(att[:, bass.ts(half, 512)], pso[:])
                    nc.sync.dma_start(out=part_d[b][bass.ts(tt, 128), :], in_=att[:])

                # ---------------- ReduceScatter + residual + LayerNorm ----------
                nc.sync.dma_start(out=dbg_part_d[b, :, :],
                                  in_=part_d[b][0:128, :])
                nc.gpsimd.collective_compute(
                    "ReduceScatter", OP.add,
                    replica_groups=[list(range(NCORES))],
                    ins=[part_d[b][:, :]], outs=[rsout_d[b][:, :]])

                y = lnp.tile([128, H], f32, tag="y")
                hr = lnp.tile([128, H], f32, tag="hr")
                nc.sync.dma_start(out=y[:], in_=rsout_d[b][:, :])
                nc.sync.dma_start(out=hr[:], in_=hres_d[b, :, :])
                nc.vector.tensor_tensor(out=y[:], in0=y[:], in1=hr[:], op=OP.add)
                negmu = sm.tile([128, 1], f32, tag="negmu")
                nc.vector.reduce_sum(negmu[:], y[:], axis=AX.X, negate=True)
                nc.vector.tensor_scalar_mul(negmu[:], negmu[:], 1.0 / H)
                z = lnp.tile([128, H], f32, tag="z")
                nc.vector.tensor_scalar_add(z[:], y[:], negmu[:])
                zsq = lnp.tile([128, H], f32, tag="zsq")
                varsum = sm.tile([128, 1], f32, tag="varsum")
                nc.scalar.activation(zsq[:], z[:], AF.Square, accum_out=varsum[:])
                sqv = sm.tile([128, 1], f32, tag="sqv")
                nc.scalar.activation(sqv[:], varsum[:], AF.Sqrt,
                                     bias=epsb[:], scale=1.0 / H)
                rstd = sm.tile([128, 1], f32, tag="rstd")
                nc.vector.reciprocal(rstd[:], sqv[:])
                nc.vector.tensor_scalar_mul(z[:], z[:], rstd[:])
                nc.vector.tensor_tensor(out=z[:], in0=z[:], in1=lnwB[:], op=OP.mult)
                nc.vector.tensor_tensor(out=z[:], in0=z[:], in1=lnbB[:], op=OP.add)
                nc.sync.dma_start(out=out_d[b, :, :], in_=z[:])

    nc.compile()
    return nc


_NC_CACHE = {}


def _get_nc(f32r=True):
    if f32r not in _NC_CACHE:
        _NC_CACHE[f32r] = build_nc(f32r)
    return _NC_CACHE[f32r]


def _prep_inputs(hidden_states, encoder_hidden_states, encoder_attention_mask,
                 q_w, q_b, k_w, k_b, v_w, v_b, o_w, o_b, ln_w, ln_b):
    f4 = np.float32
    h = np.asarray(hidden_states, f4)
    enc = np.asarray(encoder_hidden_states, f4)
    mask = np.asarray(encoder_attention_mask)
    hT = np.ascontiguousarray(h.transpose(0, 2, 1))
    encT = np.ascontiguousarray(enc.transpose(0, 2, 1))
    mb = np.where(mask == 0, np.float32(NEG), np.float32(0.0))
    # mbT[p, b*32+j] = mb[b, j*128+p]
    mbT = np.ascontiguousarray(
        mb.reshape(B, NKV, 128).transpose(2, 0, 1).reshape(128, B * NKV))
    onesr = np.ones((1, LKV), f4)
    q_w8 = np.asarray(q_w, f4) / 8.0
    q_b8 = np.asarray(q_b, f4) / 8.0
    o_b8 = np.asarray(o_b, f4) / NCORES
    owT_full = np.ascontiguousarray(np.asarray(o_w, f4).T)  # [H, H] rows = d

    in_maps = []
    for c in range(NCORES):
        sl = slice(CL * c, CL * (c + 1))
        in_maps.append({
            "hT": hT,
            "encT": encT,
            "qwT": np.ascontiguousarray(q_w8[sl, :].T),
            "kwT": np.ascontiguousarray(np.asarray(k_w, f4)[sl, :].T),
            "vwT": np.ascontiguousarray(np.asarray(v_w, f4)[sl, :].T),
            "owT": np.ascontiguousarray(owT_full[sl, :]),
            "onesr": onesr,
            "qb8": np.ascontiguousarray(q_b8[sl][None, :]),
            "kb": np.ascontiguousarray(np.asarray(k_b, f4)[sl][None, :]),
            "vb": np.ascontiguousarray(np.asarray(v_b, f4)[sl][None, :]),
            "ob8": np.ascontiguousarray(o_b8[None, :]),
            "mbT": mbT,
            "hres": np.ascontiguousarray(
                h[:, 128 * c:128 * (c + 1), :]),
            "lnw": np.ascontiguousarray(np.asarray(ln_w, f4)[None, :]),
            "lnb": np.ascontiguousarray(np.asarray(ln_b, f4)[None, :]),
        })
    return in_maps, mask


def _assemble(results, mask):
    scores = np.concatenate([r["scores_loc"] for r in results], axis=1)
    out = np.empty((B, LQ, H), np.float32)
    for c in range(NCORES):
        out[:, 128 * c:128 * (c + 1), :] = results[c]["out_loc"]
    m = np.asarray(mask)
    if (m == 0).any():
        for b in range(B):
            scores[b, :, :, m[b] == 0] = -np.inf
    return out, scores


def run(inputs, f32r=True, trace=False):
    _install_ntff_hook()
    nc = _get_nc(f32r)
    in_maps, mask = _prep_inputs(**inputs)
    res = run_bass_kernel_spmd(nc, in_maps, list(range(NCORES)), trace=trace)
    out, scores = _assemble(res.results, mask)
    return (out, scores), res


def kernel(**inputs):
    (out, scores), _ = run(inputs, f32r=True, trace=False)
    return (out, scores)
